# revision 1
# baseline (speedup 1.0000x reference)
"""Lorentz multi-head attention on 8 Trainium2 NeuronCores.

Sharding: head-parallel phase 1 (core c computes head c for all batches:
QKV Lorentz projections, Lorentz-inner-product scores, softmax-free
exp-attention, Lorentz-midpoint normalize), then an AllToAll exchanges
(head-block -> token-block) so phase 2 (concat_logradius fusion + output
LorentzFC) runs token-parallel (core c handles 1024 of the 8192 tokens).

Softmax denominator is skipped entirely: the Lorentz midpoint renormalizes
m / sqrt(K*(t^2-||s||^2)), which is invariant to positive row scaling, so
exp(scores) can be used unnormalized (scores are O(+-5), no overflow risk).

Biases are folded into the matmuls by augmenting tokens with a constant-1
column and weights with a bias row. sqrt/rsqrt are computed as
exp(+-0.5*ln(x)) so the ScalarEngine needs only the one
natural_log_exp_and_others table set (no ~2.7us table swaps).

Big matmuls run in float32r (4x the fp32 rate at moving-dim >= 256);
attention output is accumulated transposed ([65, n]) so its moving dim is
512, then rotated back 128 tokens at a time through the PE transpose path.
"""

import os
import sys

sys.path.insert(0, "/opt/trn_rl_repo")

import numpy as np

_SKIP_CC = os.environ.get("LA_SKIP_CC", "0") == "1"  # debug: phase 1 only
_SKIP_P1 = os.environ.get("LA_SKIP_P1", "0") == "1"  # debug: phase 2 only

import concourse.bass as bass
import concourse.mybir as mybir
import concourse.tile as tile
from concourse import bacc, bass_utils
from concourse.masks import make_identity

# Problem constants (hardcoded per task contract)
B, N, D = 4, 2048, 513
H, DHS = 8, 64
NCORES = 8
KCURV = 0.1
INVK = 10.0
SCALE = 1.0 / np.sqrt(DHS)  # 0.125
S_CONST = 2.8479428291320801  # exp(0.5*(digamma(256)-digamma(32)))
DPAD = 640  # 513 padded to 5*128 (col 513 = constant-1 bias lane)
KC = 5  # contraction chunks of 128
BN = B * N  # 8192 tokens
RPC = BN // NCORES  # 1024 rows per core in phase 2
F32 = mybir.dt.float32
BF16 = mybir.dt.bfloat16
Ln = mybir.ActivationFunctionType.Ln
Exp = mybir.ActivationFunctionType.Exp

_CACHE = {}




def _patch_act_tables(nc):
    # Exp and Ln both live in the natural_log_exp_and_others set; the
    # table-load pass picks the first set containing each function, which
    # splits them across two sets and reloads tables on every Ln<->Exp
    # switch (~1.3us each). Restrict the map so the combined set wins.
    from concourse.hw_specs import get_activation_tables

    try:
        tabs = get_activation_tables(nc.m.arch)
    except Exception:
        return
    if "natural_log_exp_and_others" not in tabs:
        return
    for name, fns in tabs.items():
        if name != "natural_log_exp_and_others":
            fns.discard(Exp)
            fns.discard(Ln)


def _build():
    nc = bacc.Bacc(
        "TRN2", target_bir_lowering=False, debug=False, num_devices=NCORES
    )
    _patch_act_tables(nc)

    xT_ap = nc.dram_tensor("xT", [DPAD, BN], F32, kind="ExternalInput").ap()
    wqT_ap = nc.dram_tensor("wqT", [DPAD, DHS], F32, kind="ExternalInput").ap()
    wkT_ap = nc.dram_tensor("wkT", [DPAD, DHS], F32, kind="ExternalInput").ap()
    wvT_ap = nc.dram_tensor("wvT", [DPAD, DHS], F32, kind="ExternalInput").ap()
    woT_ap = nc.dram_tensor("woT", [DPAD, D - 1], F32, kind="ExternalInput").ap()
    y_ap = nc.dram_tensor("y", [RPC, D], F32, kind="ExternalOutput").ap()

    with tile.TileContext(nc) as tc:
        with (
            tc.tile_pool(name="const", bufs=1) as constp,
            tc.tile_pool(name="w", bufs=1) as wp,
            tc.tile_pool(name="xT", bufs=5) as xtp,
            tc.tile_pool(name="qk", bufs=2) as qkp,
            tc.tile_pool(name="sq", bufs=2) as sqp,
            tc.tile_pool(name="va", bufs=2) as vap,
            tc.tile_pool(name="pt", bufs=3) as ptp,
            tc.tile_pool(name="sm", bufs=2) as smp,
            tc.tile_pool(name="d2", bufs=2) as d2p,
            tc.tile_pool(name="ps", bufs=3, space="PSUM") as psp,
            tc.tile_pool(name="acc", bufs=1, space="PSUM") as accp,
            tc.tile_pool(name="dram", bufs=1, space="DRAM") as dramp,
        ):
            ident = constp.tile([128, 128], F32)
            make_identity(nc, ident[:])
            ones65 = constp.tile([65, 1], F32)
            nc.vector.memset(ones65[:], 1.0)
            ones65b = constp.tile([65, 1], BF16)
            nc.vector.memset(ones65b[:], 1.0)
            one1 = constp.tile([1, 1], F32)
            nc.vector.memset(one1[:], 1.0)
            bias10 = constp.tile([128, 1], F32)
            nc.vector.memset(bias10[:], INVK)
            biasD = constp.tile([128, 1], F32)
            nc.vector.memset(biasD[:], INVK * (1.0 + H * S_CONST * S_CONST))

            # Weights: [DPAD, S] viewed as [128, KC, S]
            wq = wp.tile([128, KC, DHS], F32)
            wk = wp.tile([128, KC, DHS], F32)
            wv = wp.tile([128, KC, DHS], F32)
            wo = wp.tile([128, KC, D - 1], F32)
            for w_t, w_src in ((wq, wqT_ap), (wk, wkT_ap), (wv, wvT_ap)):
                nc.sync.dma_start(
                    w_t[:], w_src.rearrange("(k p) s -> p k s", p=128)
                )
            nc.sync.dma_start(wo[:], woT_ap.rearrange("(k p) s -> p k s", p=128))
            wqb = wp.tile([128, KC, DHS], BF16)
            wkb = wp.tile([128, KC, DHS], BF16)
            wvb = wp.tile([128, KC, DHS], BF16)
            wob = wp.tile([128, KC, D - 1], BF16)
            for bf_t, f_t in ((wqb, wq), (wkb, wk), (wvb, wv), (wob, wo)):
                nc.vector.tensor_copy(bf_t[:], f_t[:])

            send = dramp.tile([BN, DHS + 1], F32)
            recv = dramp.tile([BN, DHS + 1], F32)

            # ================= Phase 1: per-batch attention =================
            for b in range(B) if not _SKIP_P1 else []:
                # ---- load xT_b chunks [128, N] x 5
                xt = []
                for ki in range(KC):
                    t = xtp.tile([128, N], F32, tag="xT", bufs=3)
                    nc.sync.dma_start(
                        t[:],
                        xT_ap[ki * 128 : (ki + 1) * 128, b * N : (b + 1) * N],
                    )
                    xt.append(t)
                xtb = []
                for ki in range(KC):
                    tb = xtp.tile([128, N], BF16, tag="xTb", name=f"xb{b}_{ki}")
                    nc.vector.tensor_copy(tb[:], xt[ki][:])
                    xtb.append(tb)

                # ---- q/k projections -> [65, N] augmented (row 64 = +-t)
                qa = qkp.tile([65, N], BF16, tag="qa")
                ka = qkp.tile([65, N], BF16, tag="ka")
                for w_t, dst, neg in ((wqb, qa, False), (wkb, ka, True)):
                    for nj in range(N // 512):
                        ps = psp.tile([64, 512], F32, tag="ps")
                        for ki in range(KC):
                            nc.tensor.matmul(
                                ps[:],
                                w_t[:, ki, :],
                                xtb[ki][:, nj * 512 : (nj + 1) * 512],
                                start=(ki == 0),
                                stop=(ki == KC - 1),
                            )
                        nc.vector.tensor_copy(
                            dst[0:64, nj * 512 : (nj + 1) * 512], ps[:]
                        )
                    # t = sqrt(INVK + sum(space^2)): ones-matmul col-sum of
                    # squares, then one Ln + one Exp over the full row
                    sq = sqp.tile([64, N], BF16, tag="sq")
                    nc.vector.tensor_mul(sq[:], dst[0:64, :], dst[0:64, :])
                    srow = smp.tile([1, N], F32, tag="row", bufs=3)
                    for nj in range(N // 512):
                        pst = psp.tile([1, 512], F32, tag="ps")
                        nc.tensor.matmul(
                            pst[:],
                            ones65b[0:64, :],
                            sq[:, nj * 512 : (nj + 1) * 512],
                            start=True,
                            stop=True,
                        )
                        nc.vector.tensor_copy(
                            srow[:, nj * 512 : (nj + 1) * 512], pst[:]
                        )
                    lrow = smp.tile([1, N], F32, tag="row", bufs=3)
                    nc.scalar.activation(lrow[:], srow[:], Ln, bias=bias10[0:1, :])
                    if neg:
                        # k gets -t so the scores matmul computes the Lorentz
                        # product q.k - t_q*t_k in one pass
                        trow = smp.tile([1, N], F32, tag="row", bufs=3)
                        nc.scalar.activation(trow[:], lrow[:], Exp, scale=0.5)
                        nc.scalar.mul(dst[64:65, :], trow[:], -1.0)
                    else:
                        nc.scalar.activation(dst[64:65, :], lrow[:], Exp, scale=0.5)

                # ---- v projection, natural layout [128, mi, 65] (col0 = t)
                va = vap.tile([128, N // 128, DHS + 1], BF16, tag="va")
                vts = smp.tile([128, N // 128, 1], F32, tag="vts")
                for mi in range(N // 128):
                    psv = psp.tile([128, 64], F32, tag="ps")
                    for ki in range(KC):
                        nc.tensor.matmul(
                            psv[:],
                            xtb[ki][:, mi * 128 : (mi + 1) * 128],
                            wvb[:, ki, :],
                            start=(ki == 0),
                            stop=(ki == KC - 1),
                        )
                    nc.vector.tensor_copy(va[:, mi, 1:65], psv[:])
                    vsq = smp.tile([128, 64], F32, tag="vsq")
                    nc.vector.tensor_mul(vsq[:], va[:, mi, 1:65], va[:, mi, 1:65])
                    nc.vector.reduce_sum(
                        vts[:, mi, :], vsq[:], axis=mybir.AxisListType.X
                    )
                # batched t_v = exp(.5 ln(sum + INVK)) for all 16 chunks
                lnv = smp.tile([128, N // 128, 1], F32, tag="lnv")
                nc.scalar.activation(lnv[:], vts[:], Ln, bias=bias10[:])
                nc.scalar.activation(va[:, :, 0:1], lnv[:], Exp, scale=0.5)

                # ---- attention: scores^T -> exp -> m^T accumulation (f32r)
                mts = []
                for nj in range(N // 512):
                    mtile = accp.tile([65, 512], F32, tag=f"acc{nj}",
                                      name=f"mts{b}_{nj}")
                    mts.append(mtile)
                for mi in range(N // 128):
                    pt = ptp.tile([128, N], BF16, tag="pt")
                    for nj in range(N // 512):
                        pss = psp.tile([128, 512], F32, tag="ps")
                        nc.tensor.matmul(
                            pss[:],
                            ka[:, mi * 128 : (mi + 1) * 128],
                            qa[:, nj * 512 : (nj + 1) * 512],
                            start=True,
                            stop=True,
                        )
                        nc.scalar.activation(
                            pt[:, nj * 512 : (nj + 1) * 512], pss[:], Exp,
                            scale=SCALE,
                        )
                    for nj in range(N // 512):
                        nc.tensor.matmul(
                            mts[nj][:],
                            va[:, mi, :],
                            pt[:, nj * 512 : (nj + 1) * 512],
                            start=(mi == 0),
                            stop=(mi == N // 128 - 1),
                        )

                # ---- Lorentz midpoint normalize (transposed layout)
                mT = sqp.tile([65, N], F32, tag="mt")
                for nj in range(N // 512):
                    nc.vector.tensor_copy(mT[:, nj * 512 : (nj + 1) * 512],
                                          mts[nj][:])
                sqT = sqp.tile([65, N], F32, tag="sq")
                nc.vector.tensor_mul(sqT[:], mT[:], mT[:])
                rT = smp.tile([1, N], F32, tag="row", bufs=3)
                for nj in range(N // 512):
                    psc = psp.tile([1, 512], F32, tag="ps")
                    nc.tensor.matmul(
                        psc[:],
                        ones65[:],
                        sqT[:, nj * 512 : (nj + 1) * 512],
                        start=True,
                        stop=True,
                    )
                    # r = 2*t^2 - sum_all(sq)  (= t^2 - ||space||^2)
                    t2c = smp.tile([1, 512], F32, tag="t2")
                    nc.vector.tensor_scalar_mul(
                        t2c[:], sqT[0:1, nj * 512 : (nj + 1) * 512], 2.0
                    )
                    nc.vector.tensor_sub(
                        rT[:, nj * 512 : (nj + 1) * 512], t2c[:], psc[:]
                    )
                # rotate r into token-partition layout via K=1 matmuls,
                # then one Ln + one Exp for all 16 chunks
                prl = psp.tile([128, N // 128], F32, tag="pr", bufs=1)
                for j in range(N // 128):
                    nc.tensor.matmul(
                        prl[:, j : j + 1],
                        rT[:, j * 128 : (j + 1) * 128],
                        one1[:],
                        start=True,
                        stop=True,
                    )
                lnr = smp.tile([128, N // 128], F32, tag="lnr")
                nc.scalar.activation(lnr[:], prl[:], Ln, scale=KCURV)
                rinv = smp.tile([128, N // 128], F32, tag="rinv")
                nc.scalar.activation(rinv[:], lnr[:], Exp, scale=-0.5)
                for nj2 in range(N // 128):
                    ptr2 = psp.tile([128, 65], F32, tag="ps")
                    nc.tensor.transpose(
                        ptr2[:], mT[:, nj2 * 128 : (nj2 + 1) * 128],
                        ident[0:65, 0:65],
                    )
                    mo = smp.tile([128, DHS + 1], F32, tag="mo", bufs=4)
                    nc.vector.tensor_scalar_mul(
                        mo[:], ptr2[:], rinv[:, nj2 : nj2 + 1]
                    )
                    nc.sync.dma_start(
                        send[b * N + nj2 * 128 : b * N + (nj2 + 1) * 128, :],
                        mo[:],
                    )

            if _SKIP_CC:
                dbg = d2p.tile([128, DHS + 1], F32, tag="rv")
                for r in range(RPC // 128):
                    nc.sync.dma_start(dbg[:], send[r * 128 : (r + 1) * 128, :])
                    nc.sync.dma_start(
                        y_ap[r * 128 : (r + 1) * 128, 0 : DHS + 1], dbg[:]
                    )
            else:
                # ============ Phase 2: exchange + fusion + out proj =========
                nc.gpsimd.collective_compute(
                    "AllToAll",
                    mybir.AluOpType.bypass,
                    replica_groups=[list(range(NCORES))],
                    ins=[send.opt()],
                    outs=[recv.opt()],
                )
                # recv rows: j*1024 + q*128 + p  (j = head, q = row chunk)
                recv_r = recv[:].rearrange(
                    "(j q p) d -> q p j d", j=H, q=8, p=128
                )

                rvs = []
                tsA = smp.tile([128, RPC // 128], F32, tag="tsA")
                for r in range(RPC // 128):
                    rv = d2p.tile([128, H, DHS + 1], F32, tag="rv", bufs=8,
                                  name=f"rv{r}")
                    nc.sync.dma_start(rv[:], recv_r[r])
                    rvs.append(rv)
                    tsq = smp.tile([128, H, 1], F32, tag="tsq")
                    nc.vector.tensor_mul(tsq[:], rv[:, :, 0:1], rv[:, :, 0:1])
                    nc.vector.reduce_sum(
                        tsA[:, r : r + 1], tsq[:, :, 0],
                        axis=mybir.AxisListType.X,
                    )
                # t' = exp(.5 ln(s^2 * sum_h t_h^2 + INVK*(1+H*s^2))),
                # batched over all 8 row chunks
                lnt2 = smp.tile([128, RPC // 128], F32, tag="lnt2")
                nc.scalar.activation(
                    lnt2[:], tsA[:], Ln, scale=S_CONST * S_CONST, bias=biasD[:]
                )
                tpA = smp.tile([128, RPC // 128], F32, tag="tpA")
                nc.scalar.activation(tpA[:], lnt2[:], Exp, scale=0.5)

                outts = []
                osA = smp.tile([128, RPC // 128], F32, tag="osA")
                for r in range(RPC // 128):
                    rv = rvs[r]
                    fu = d2p.tile([128, DPAD], F32, tag="fu", bufs=1)
                    nc.vector.tensor_copy(fu[:, 0:1], tpA[:, r : r + 1])
                    nc.vector.tensor_scalar_mul(
                        fu[:, 1:513].rearrange("p (j s) -> p j s", j=H),
                        rv[:, :, 1:65],
                        S_CONST,
                    )
                    nc.vector.memset(fu[:, 513:514], 1.0)
                    nc.vector.memset(fu[:, 514:DPAD], 0.0)

                    # transpose to [d, tokens] for the output contraction
                    ft = d2p.tile([128, KC, 128], BF16, tag="ft")
                    for ki in range(KC):
                        pstr = psp.tile([128, 128], F32, tag="ps")
                        nc.tensor.transpose(
                            pstr[:], fu[:, ki * 128 : (ki + 1) * 128], ident[:]
                        )
                        nc.vector.tensor_copy(ft[:, ki, :], pstr[:])

                    # output projection [128 tokens, 512]
                    pso = psp.tile([128, 512], F32, tag="ps")
                    for ki in range(KC):
                        nc.tensor.matmul(
                            pso[:],
                            ft[:, ki, :],
                            wob[:, ki, :],
                            start=(ki == 0),
                            stop=(ki == KC - 1),
                        )
                    outt = d2p.tile([128, D], F32, tag="out", bufs=8,
                                    name=f"outt{r}")
                    nc.vector.tensor_copy(outt[:, 1:D], pso[:])
                    outts.append(outt)
                    osq = smp.tile([128, 512], F32, tag="osq")
                    nc.vector.tensor_mul(osq[:], outt[:, 1:D], outt[:, 1:D])
                    nc.vector.reduce_sum(
                        osA[:, r : r + 1], osq[:], axis=mybir.AxisListType.X
                    )
                # batched t_out = exp(.5 ln(sum + INVK)), then store
                lno = smp.tile([128, RPC // 128], F32, tag="lno")
                nc.scalar.activation(lno[:], osA[:], Ln, bias=bias10[:])
                toA = smp.tile([128, RPC // 128], F32, tag="toA")
                nc.scalar.activation(toA[:], lno[:], Exp, scale=0.5)
                for r in range(RPC // 128):
                    nc.vector.tensor_copy(
                        outts[r][:, 0:1], toA[:, r : r + 1]
                    )
                    nc.sync.dma_start(
                        y_ap[r * 128 : (r + 1) * 128, :], outts[r][:]
                    )

    nc.compile()
    return nc


def _prep_inputs(x, Wq, bq, Wk, bk, Wv, bv, Wo, bo):
    xT = np.zeros((DPAD, BN), dtype=np.float32)
    xT[:D, :] = np.ascontiguousarray(x.reshape(BN, D).T)
    xT[D, :] = 1.0

    woT = np.zeros((DPAD, D - 1), dtype=np.float32)
    woT[:D + 1, :] = np.concatenate([Wo.T, bo[None, :]], axis=0)

    in_maps = []
    for h in range(NCORES):
        m = {"xT": xT, "woT": woT}
        for nm, W, bvec in (("wqT", Wq, bq), ("wkT", Wk, bk), ("wvT", Wv, bv)):
            w = np.zeros((DPAD, DHS), dtype=np.float32)
            w[0:D + 1, :] = np.concatenate([W[h].T, bvec[h][None, :]], axis=0)
            m[nm] = w
        in_maps.append(m)
    return in_maps


def _run(inputs, trace=False, **kw):
    if "nc" not in _CACHE:
        _CACHE["nc"] = _build()
    nc = _CACHE["nc"]
    in_maps = _prep_inputs(**{k: np.asarray(v) for k, v in inputs.items()})
    res = bass_utils.run_bass_kernel_spmd(
        nc, in_maps, core_ids=list(range(NCORES)), trace=trace, **kw
    )
    y = np.concatenate([res.results[c]["y"] for c in range(NCORES)], axis=0)
    return y.reshape(B, N, D), res


def kernel(**inputs):
    y, _ = _run(inputs)
    return y

